# revision 56
# baseline (speedup 1.0000x reference)
"""Trainium2 Bass kernel for nn_AssociatorLoss.

Reference (B=32, N=32), a = cayley_cube (B,N,N,N):
    one[b,i,j,k,l] = sum_m a[b,i,m,l] * a[b,j,k,m]
    two[b,i,j,k,l] = sum_m a[b,m,k,l] * a[b,i,j,m]
    kl = sum(two * (log(two) - log(one))) / B

Data-parallel over b (4 per core, 8 cores, host combines partial sums;
no collectives).

Per batch element b, chunk c covers i in [4c,4c+4).  The two einsums
are K=32 x F=512 bf16 matmuls:
    tp = two chunk  [p=(di,j), f]     op = one chunk  [p=(di,l), f]
with all operand layouts precomputed on the host (stacked into stq/mvq;
device does no prep).  Column order of both products is paired:
f = kH*64 + x*2 + kL (x = l for two / j for one, k = 2kH+kL), chosen so
that viewing a bf16 tensor as uint32 pairs and running the 32x32-block
StreamTranspose performs exactly the one-layout -> two-layout partition
alignment at half the element count.

Per chunk (log-difference pipeline):
    lo  = Ln(1024*op)   ACT   (evacuates op from PSUM, bf16)
    lt  = Ln(1024*tp)   ACT   (evacuates tp, two-layout)
    loT = pairT(lo)     DVE   (uint32 StreamTranspose -> two-layout)
    dd  = lt - loT      DVE / Pool alternating (bf16; the Ln scale
                         offsets cancel in the difference)
    acc[:, chunk] = sum(tp * dd)   DVE stt with accumulate
kl = sum(acc) / B on the host in float64.
"""

import sys

for _p in ("/opt/trn_rl_repo",):
    if _p not in sys.path:
        sys.path.insert(0, _p)

import ml_dtypes
import numpy as np

import concourse.bacc as bacc
import concourse.mybir as mybir
import concourse.tile as tile
from concourse.bass_utils import run_bass_kernel_spmd

B, N = 32, 32
N_CORES = 8
B_LOCAL = B // N_CORES      # 4
NCHUNK = 8                  # chunks of 128 (i,*) rows per batch element
NGROUP = B_LOCAL * NCHUNK // 4   # 8 groups of 4 chunks
F32 = mybir.dt.float32
BF16 = mybir.dt.bfloat16
LN_SCALE = 1024.0           # centers ln() inputs near 0 for bf16 precision

def _bf16(x):
    return np.ascontiguousarray(x).astype(ml_dtypes.bfloat16)


def host_prep(a_local: np.ndarray):
    """a_local [B_LOCAL, N, N, N] f32 -> operand dict (per-core inputs)."""
    out = {}
    for b in range(B_LOCAL):
        A = np.ascontiguousarray(a_local[b], dtype=np.float32)
        st2 = A.transpose(2, 0, 1).reshape(N, N * N)    # [m,(i,j)]
        st1 = A.transpose(1, 0, 2).reshape(N, N * N)    # [m,(i,l)]
        # paired column orders: col = kH*64 + x*2 + kL
        mv2p = A.reshape(N, 16, 2, N).transpose(0, 1, 3, 2).reshape(N, N * N)
        mv1p = (A.transpose(2, 1, 0).reshape(N, 16, 2, N)
                .transpose(0, 1, 3, 2).reshape(N, N * N))
        out[f"stq_{b}"] = _bf16(np.concatenate([st1, st2], axis=0))
        out[f"mvq_{b}"] = _bf16(np.concatenate([mv1p, mv2p], axis=0))
    return out


def build():
    nc = bacc.Bacc(None, target_bir_lowering=False)
    mult = mybir.AluOpType.mult
    subtract = mybir.AluOpType.subtract
    Ln = mybir.ActivationFunctionType.Ln

    ext = {}
    for b in range(B_LOCAL):
        for nm, shape in (("stq", [64, N * N]), ("mvq", [64, N * N])):
            ext[f"{nm}_{b}"] = nc.declare_dram_parameter(
                f"{nm}_{b}", shape, BF16, isOutput=False)
    out_ext = nc.declare_dram_parameter("out", [128, 32], F32, isOutput=True)

    with tile.TileContext(nc) as tc:
        with (
            tc.tile_pool(name="apool", bufs=2) as apool,
            tc.tile_pool(name="spool", bufs=6) as spool,
            tc.tile_pool(name="scratch", bufs=1) as scratch,
            tc.tile_pool(name="accp", bufs=1) as accpool,
            tc.tile_pool(name="psumO", bufs=2, space="PSUM") as psumO,
            tc.tile_pool(name="psumT", bufs=2, space="PSUM") as psumT,
        ):
            acc = accpool.tile([128, 32], F32)
            p3 = scratch.tile([128, 1024], BF16)
            p4 = scratch.tile([128, 1024], BF16)

            # prefetch all batch elements' operands up front
            tensors = []
            for b in range(B_LOCAL):
                t = {}
                for nm, shape in (("stq", [64, N * N]), ("mvq", [64, N * N])):
                    tt = apool.tile(shape, BF16, tag=nm, name=nm, bufs=B_LOCAL)
                    if b == 0:
                        # rows 0:32 (one-product operands) land first so the
                        # first matmuls start as early as possible
                        nc.sync.dma_start(out=tt[0:32, 0:512],
                                          in_=ext[f"{nm}_{b}"][0:32, 0:512])
                        nc.sync.dma_start(out=tt[0:32, 512:1024],
                                          in_=ext[f"{nm}_{b}"][0:32, 512:1024])
                        nc.sync.dma_start(out=tt[32:64, :],
                                          in_=ext[f"{nm}_{b}"][32:64, :])
                    else:
                        nc.sync.dma_start(out=tt[:], in_=ext[f"{nm}_{b}"][:])
                    t[nm] = tt
                tensors.append(t)

            chunk_idx = 0
            for b in range(B_LOCAL):
                t = tensors[b]
                for c in range(NCHUNK):
                    ms = slice(128 * c, 128 * (c + 1))
                    op = psumO.tile([128, 1024], F32, tag="op", name="op")
                    tp = psumT.tile([128, 1024], F32, tag="tp", name="tp")
                    for h in range(2):
                        cs = slice(512 * h, 512 * (h + 1))
                        nc.tensor.matmul(op[:, cs], t["stq"][0:32, ms],
                                         t["mvq"][0:32, cs], start=True,
                                         stop=True)
                    for h in range(2):
                        cs = slice(512 * h, 512 * (h + 1))
                        nc.tensor.matmul(tp[:, cs], t["stq"][32:64, ms],
                                         t["mvq"][32:64, cs], start=True,
                                         stop=True)

                    lo = spool.tile([128, 1024], BF16, tag="lo")
                    nc.scalar.activation(lo[:], op[:], Ln, scale=LN_SCALE)
                    lt = spool.tile([128, 1024], BF16, tag="lt")
                    nc.scalar.activation(lt[:], tp[:], Ln, scale=LN_SCALE)
                    loT = spool.tile([128, 1024], BF16, tag="loT")
                    nc.vector.transpose(
                        loT[:].bitcast(mybir.dt.uint32),
                        lo[:].bitcast(mybir.dt.uint32))
                    # dd split by columns: Pool (slow, off critical DVE path)
                    # takes the wide piece, DVE the narrow one, in parallel
                    dd = spool.tile([128, 1024], BF16, tag="dd")
                    nc.gpsimd.tensor_tensor(out=dd[:, 0:704], in0=lt[:, 0:704],
                                            in1=loT[:, 0:704], op=subtract)
                    nc.vector.tensor_tensor(out=dd[:, 704:1024],
                                            in0=lt[:, 704:1024],
                                            in1=loT[:, 704:1024], op=subtract)
                    # dot on DVE: acc col = sum(tp * dd)
                    nc.vector.scalar_tensor_tensor(
                        out=p3[:], in0=tp[:], scalar=1.0,
                        in1=dd[:], op0=mult, op1=mult,
                        accum_out=acc[:, chunk_idx:chunk_idx + 1])
                    chunk_idx += 1

            nc.sync.dma_start(out=out_ext[:, :], in_=acc[:])

    nc.compile()
    return nc


def kernel(cayley_cube: np.ndarray) -> np.ndarray:
    assert cayley_cube.shape == (B, N, N, N)
    nc = build()
    shards = cayley_cube.reshape(N_CORES, B_LOCAL, N, N, N)
    in_maps = [host_prep(shards[i]) for i in range(N_CORES)]
    res = run_bass_kernel_spmd(nc, in_maps, core_ids=list(range(N_CORES)))
    tot = np.float64(0.0)
    for r in res.results:
        tot += r["out"].sum(dtype=np.float64)
    return np.float32(tot / B)


if __name__ == "__main__":
    rng = np.random.default_rng(0)
    raw = rng.uniform(0.05, 1.0, size=(B, N, N, N)).astype(np.float32)
    a = raw / raw.sum(axis=-1, keepdims=True)
    print(kernel(a))


# revision 57
# speedup vs baseline: 1.2045x; 1.2045x over previous
"""Trainium2 Bass kernel for nn_AssociatorLoss.

Reference (B=32, N=32), a = cayley_cube (B,N,N,N):
    one[b,i,j,k,l] = sum_m a[b,i,m,l] * a[b,j,k,m]
    two[b,i,j,k,l] = sum_m a[b,m,k,l] * a[b,i,j,m]
    kl = sum(two * (log(two) - log(one))) / B

Data-parallel over b (4 per core, 8 cores, host combines partial sums;
no collectives).

Per batch element b, chunk c covers i in [4c,4c+4).  The two einsums
are K=32 x F=512 bf16 matmuls:
    tp = two chunk  [p=(di,j), f]     op = one chunk  [p=(di,l), f]
with all operand layouts precomputed on the host (stacked into stq/mvq;
device does no prep).  Column order of both products is paired:
f = kH*64 + x*2 + kL (x = l for two / j for one, k = 2kH+kL), chosen so
that viewing a bf16 tensor as uint32 pairs and running the 32x32-block
StreamTranspose performs exactly the one-layout -> two-layout partition
alignment at half the element count.

Per chunk (log-difference pipeline):
    lo  = Ln(1024*op)   ACT   (evacuates op from PSUM, bf16)
    lt  = Ln(1024*tp)   ACT   (evacuates tp, two-layout)
    loT = pairT(lo)     DVE   (uint32 StreamTranspose -> two-layout)
    dd  = lt - loT      DVE / Pool alternating (bf16; the Ln scale
                         offsets cancel in the difference)
    acc[:, chunk] = sum(tp * dd)   DVE stt with accumulate
kl = sum(acc) / B on the host in float64.
"""

import sys

for _p in ("/opt/trn_rl_repo",):
    if _p not in sys.path:
        sys.path.insert(0, _p)

import ml_dtypes
import numpy as np

import concourse.bacc as bacc
import concourse.mybir as mybir
import concourse.tile as tile
from concourse.bass_utils import run_bass_kernel_spmd

B, N = 32, 32
N_CORES = 8
B_LOCAL = B // N_CORES      # 4
NCHUNK = 8                  # chunks of 128 (i,*) rows per batch element
NGROUP = B_LOCAL * NCHUNK // 4   # 8 groups of 4 chunks
F32 = mybir.dt.float32
BF16 = mybir.dt.bfloat16
LN_SCALE = 1024.0           # centers ln() inputs near 0 for bf16 precision

def _bf16(x):
    return np.ascontiguousarray(x).astype(ml_dtypes.bfloat16)


def host_prep(a_local: np.ndarray):
    """a_local [B_LOCAL, N, N, N] f32 -> operand dict (per-core inputs)."""
    out = {}
    for b in range(B_LOCAL):
        A = np.ascontiguousarray(a_local[b], dtype=np.float32)
        st2 = A.transpose(2, 0, 1).reshape(N, N * N)    # [m,(i,j)]
        st1 = A.transpose(1, 0, 2).reshape(N, N * N)    # [m,(i,l)]
        # paired column orders: col = kH*64 + x*2 + kL
        mv2p = A.reshape(N, 16, 2, N).transpose(0, 1, 3, 2).reshape(N, N * N)
        mv1p = (A.transpose(2, 1, 0).reshape(N, 16, 2, N)
                .transpose(0, 1, 3, 2).reshape(N, N * N))
        out[f"stq_{b}"] = _bf16(np.concatenate([st1, st2], axis=0))
        out[f"mvq_{b}"] = _bf16(np.concatenate([mv1p, mv2p], axis=0))
    return out


def build():
    nc = bacc.Bacc(None, target_bir_lowering=False)
    mult = mybir.AluOpType.mult
    subtract = mybir.AluOpType.subtract
    Ln = mybir.ActivationFunctionType.Ln

    ext = {}
    for b in range(B_LOCAL):
        for nm, shape in (("stq", [64, N * N]), ("mvq", [64, N * N])):
            ext[f"{nm}_{b}"] = nc.declare_dram_parameter(
                f"{nm}_{b}", shape, BF16, isOutput=False)
    out_ext = nc.declare_dram_parameter("out", [128, 32], F32, isOutput=True)

    with tile.TileContext(nc) as tc:
        with (
            tc.tile_pool(name="apool", bufs=2) as apool,
            tc.tile_pool(name="spool", bufs=6) as spool,
            tc.tile_pool(name="scratch", bufs=1) as scratch,
            tc.tile_pool(name="accp", bufs=1) as accpool,
            tc.tile_pool(name="psumO", bufs=2, space="PSUM") as psumO,
            tc.tile_pool(name="psumT", bufs=2, space="PSUM") as psumT,
        ):
            acc = accpool.tile([128, 32], F32)
            p3 = scratch.tile([128, 1024], BF16)
            p4 = scratch.tile([128, 1024], BF16)

            # prefetch all batch elements' operands up front
            tensors = []
            for b in range(B_LOCAL):
                t = {}
                for nm, shape in (("stq", [64, N * N]), ("mvq", [64, N * N])):
                    tt = apool.tile(shape, BF16, tag=nm, name=nm, bufs=B_LOCAL)
                    if b == 0:
                        # rows 0:32 (one-product operands) land first so the
                        # first matmuls start as early as possible
                        nc.sync.dma_start(out=tt[0:32, 0:512],
                                          in_=ext[f"{nm}_{b}"][0:32, 0:512])
                        nc.sync.dma_start(out=tt[0:32, 512:1024],
                                          in_=ext[f"{nm}_{b}"][0:32, 512:1024])
                        nc.sync.dma_start(out=tt[32:64, :],
                                          in_=ext[f"{nm}_{b}"][32:64, :])
                    else:
                        nc.sync.dma_start(out=tt[:], in_=ext[f"{nm}_{b}"][:])
                    t[nm] = tt
                tensors.append(t)

            chunk_idx = 0
            for b in range(B_LOCAL):
                t = tensors[b]
                for c in range(NCHUNK):
                    ms = slice(128 * c, 128 * (c + 1))
                    op = psumO.tile([128, 1024], F32, tag="op", name="op")
                    tp = psumT.tile([128, 1024], F32, tag="tp", name="tp")
                    for h in range(2):
                        cs = slice(512 * h, 512 * (h + 1))
                        nc.tensor.matmul(op[:, cs], t["stq"][0:32, ms],
                                         t["mvq"][0:32, cs], start=True,
                                         stop=True)
                    for h in range(2):
                        cs = slice(512 * h, 512 * (h + 1))
                        nc.tensor.matmul(tp[:, cs], t["stq"][32:64, ms],
                                         t["mvq"][32:64, cs], start=True,
                                         stop=True)

                    lo = spool.tile([128, 1024], BF16, tag="lo")
                    nc.scalar.activation(lo[:], op[:], Ln, scale=LN_SCALE)
                    lt = spool.tile([128, 1024], BF16, tag="lt")
                    nc.scalar.activation(lt[:], tp[:], Ln, scale=LN_SCALE)
                    loT = spool.tile([128, 1024], BF16, tag="loT")
                    nc.vector.transpose(
                        loT[:].bitcast(mybir.dt.uint32),
                        lo[:].bitcast(mybir.dt.uint32))
                    # dd split by columns: Pool (slow, off critical DVE path)
                    # takes the wide piece, DVE the narrow one, in parallel
                    dd = spool.tile([128, 1024], BF16, tag="dd")
                    nc.gpsimd.tensor_tensor(out=dd[:, 0:640], in0=lt[:, 0:640],
                                            in1=loT[:, 0:640], op=subtract)
                    nc.vector.tensor_tensor(out=dd[:, 640:1024],
                                            in0=lt[:, 640:1024],
                                            in1=loT[:, 640:1024], op=subtract)
                    # dot on DVE: acc col = sum(tp * dd)
                    nc.vector.scalar_tensor_tensor(
                        out=p3[:], in0=tp[:], scalar=1.0,
                        in1=dd[:], op0=mult, op1=mult,
                        accum_out=acc[:, chunk_idx:chunk_idx + 1])
                    chunk_idx += 1

            nc.sync.dma_start(out=out_ext[:, :], in_=acc[:])

    nc.compile()
    return nc


def kernel(cayley_cube: np.ndarray) -> np.ndarray:
    assert cayley_cube.shape == (B, N, N, N)
    nc = build()
    shards = cayley_cube.reshape(N_CORES, B_LOCAL, N, N, N)
    in_maps = [host_prep(shards[i]) for i in range(N_CORES)]
    res = run_bass_kernel_spmd(nc, in_maps, core_ids=list(range(N_CORES)))
    tot = np.float64(0.0)
    for r in res.results:
        tot += r["out"].sum(dtype=np.float64)
    return np.float32(tot / B)


if __name__ == "__main__":
    rng = np.random.default_rng(0)
    raw = rng.uniform(0.05, 1.0, size=(B, N, N, N)).astype(np.float32)
    a = raw / raw.sum(axis=-1, keepdims=True)
    print(kernel(a))
